# revision 28
# baseline (speedup 1.0000x reference)
"""MultiPositionTransfer kernel for 8 TRN2 NeuronCores (Bass/Tile).

Computes out[t,b,:] = outputs[t,b,:] @ table[min(positions[t,b], 8)] for
positions [512,32] int, outputs [512,32,128] f32, table [9,128,128] f32.
Sharding: data-parallel over T across 8 cores (2048 vectors per core);
the small table is replicated.

Per-core algorithm — count-specialized sorted GEMM:

The host buckets all T*B vectors by clipped position and deals each
bucket round-robin across the 8 cores, so per-core bucket counts are
ceil(global/8) and every core runs the same program (SPMD) with minimal
padding (npad <= 2048+9 columns for any input).  Each core's columns
are laid out slot-by-slot (buckets ordered big-first); the device runs
9 plain fp16 matmuls, one per slot, against the resident [d, 9*d]
table — no masks, no per-bucket elementwise work, no transposes (X is
sent host-transposed as [d, n]).  Slot matmuls are split on a
<=512-column PSUM-bank grid; each grid block accumulates in its own
rotating PSUM bank and is copied to SBUF fp16 by DVE/Act (alternating
whole blocks) before being DMAed out.  The host inverts the
permutation on the way back.

The table and X are concatenated into one DRAM input whose column
order follows first use (the rows a block newly needs precede its X
columns); the input chunk DMAs are hoisted to the very front of the SP
stream to hide the DMA front-end latency, and the teardown is trimmed
to the output-DMA completion waits (no drains / final barrier / sem
cleanup — the runtime resets semaphore state at NEFF load, verified on
hardware by the repeated-run test).  The schedule (input chunking,
PSUM grid, copy-engine split, output grouping/issue engines) is a
tunable CFG, auto-tuned against the instruction-cost timeline model.
The program depends only on the 9 slot widths; it is compiled once per
width tuple and cached (an unusual position distribution just triggers
one recompile and stays correct).
"""

import numpy as np
from contextlib import ExitStack

import concourse.bass as bass
import concourse.tile as tile
from concourse import mybir
from concourse.bass_utils import run_bass_kernel_spmd
from concourse.vector_clock import ScopedClock, VectorClock

P = 128
D = 128
NBUCKET = 9
F16 = mybir.dt.float16
F32 = mybir.dt.float32
N_CORES = 8
BLK = 512  # PSUM bank width in f32 columns

# schedule configuration (see tune.py)
CFG = dict(
    grid=("frac", (448, 512, 480, 480, 283)),  # PSUM grid block sizes
    inchunks=(2, 1, 1, 1),  # input DMA chunks as block-group sizes
    rows_mode="need",  # "need": rows before first-needing block; "front": all first
    copy_mode="half",  # "half": DVE+Act split; "alt": alternate full copies
    copy_frac=(1.0, 0.0, 1.0, 0.0, 1.0),  # DVE share per block copy
    last_full_dve=True,  # last block copied entirely by DVE
    hoist="front",     # hoist input DMAs: "barrier" | "front"
    y_groups=(1, 2, 2),       # blocks per y DMA, last group gets the rest
    y_engines=("scalar", "sync", "sync"),  # issue engine per y group
    warmup=0,          # dummy 128-col PE matmuls before the real work
)


def _drain_and_barrier_trimmed(self, tick_clock, wait_clock):
    nc = self.nc
    vec = tick_clock.global_clock
    for proc in range(len(vec)):
        if vec[proc] <= 0:
            continue
        unit = VectorClock([vec[p] if p == proc else 0 for p in range(len(vec))])
        nop_inst = nc.sync.nop()
        wait_clock.add_sem_waits(nop_inst.ins, ScopedClock({None: unit}))
    assert self.sems is not None
    popped = nc._tile_sem_poison_stack.pop()
    assert popped is self._sem_poison


def _install_tile_compat():
    tile.TileContext._drain_and_barrier = _drain_and_barrier_trimmed


def _split_multi_waits(nc):
    for fn in nc.m.functions:
        for bb in fn.blocks:
            insts = bb.instructions
            for i in range(len(insts) - 1, -1, -1):
                inst = insts[i]
                si = inst.sync_info
                if si is None:
                    continue
                waits = list(si.on_wait)
                cap = 0 if inst.opcode == "Drain" else 1
                if len(waits) <= cap:
                    continue
                keep = waits[len(waits) - cap:] if cap else []
                hoist = waits[: len(waits) - cap] if cap else waits
                nops = []
                for k, w in enumerate(hoist):
                    nops.append(mybir.InstNoOp(
                        name=f"{inst.name}-wsplit{k}",
                        engine=inst.engine,
                        sync_info=mybir.SyncInfo(on_wait=[w], on_update=[]),
                        bass_nofuse=True,
                    ))
                inst.sync_info = mybir.SyncInfo(
                    on_wait=keep, on_update=list(si.on_update))
                insts[i:i] = nops


def _hoist_before_entry_barrier(nc, inst_names, mode="barrier"):
    """Move the named (wait-free) instructions into the preamble block,
    just before the SP entry barrier."""
    targets = set(inst_names)
    fn = nc.m.functions[0]
    moved = []
    for bb in fn.blocks:
        insts = bb.instructions
        keep = []
        for inst in insts:
            if inst.name in targets:
                assert not (inst.sync_info and inst.sync_info.on_wait), inst.name
                moved.append(inst)
            else:
                keep.append(inst)
        if len(keep) != len(insts):
            insts[:] = keep
    assert len(moved) == len(targets), (len(moved), len(targets))
    bb0 = fn.blocks[0]
    bar = None
    for i, inst in enumerate(bb0.instructions):
        if mode == "front" and inst.engine == mybir.EngineType.SP:
            bar = i
            break
        if (inst.opcode == "EventSemaphore"
                and inst.engine == mybir.EngineType.SP):
            bar = i
            break
    assert bar is not None, "entry barrier not found"
    bb0.instructions[bar:bar] = moved


def plan(widths, cfg):
    """Shared host/device layout plan for a width tuple + schedule cfg."""
    widths = [int(w) for w in widths]
    npad = sum(widths)
    offs = np.cumsum([0] + widths)

    # PSUM grid
    if isinstance(cfg["grid"], tuple) and cfg["grid"][0] == "frac":
        # relative block sizes, scaled to npad, each capped at BLK
        rel = np.array(cfg["grid"][1], dtype=float)
        sizes = np.floor(rel / rel.sum() * npad).astype(int)
        sizes[-1] += npad - sizes.sum()
        while sizes.max() > BLK or sizes.min() <= 0:
            sizes = None
            break
        if sizes is None:
            sizes = []
            c = npad
            while c > 0:
                sizes.append(min(c, BLK))
                c -= min(c, BLK)
        cuts = np.cumsum([0] + list(sizes))
        grid = list(zip(cuts[:-1].tolist(), cuts[1:].tolist()))
    else:
        grid = []
        c = 0
        while c < npad:
            grid.append((c, min(npad, c + BLK)))
            c += BLK
        if isinstance(cfg["grid"], tuple) and cfg["grid"][0] == "tiny":
            t = cfg["grid"][1]
            a, b = grid[-1]
            if b - a > t:
                grid[-1] = (a, b - t)
                grid.append((b - t, b))

    # slot matmul pieces split on the grid
    nslot = len(widths)
    pieces = []
    for s in range(nslot):
        c, b = int(offs[s]), int(offs[s + 1])
        for (ga, gb) in grid:
            lo, hi = max(c, ga), min(b, gb)
            if lo < hi:
                pieces.append((lo, hi, s))

    nblk = len(grid)
    # concat layout: rows (per cfg) interleaved with per-block X columns
    row_pos = [None] * nslot
    xcol0 = [None] * nblk
    pos = 0
    if cfg["rows_mode"] == "front":
        for (c, e, s) in pieces:
            if row_pos[s] is None:
                row_pos[s] = pos
                pos += D
    bounds = [0]
    for blk, (a, b) in enumerate(grid):
        if cfg["rows_mode"] == "need":
            for (c, e, s) in pieces:
                if c >= a and c < b and row_pos[s] is None:
                    row_pos[s] = pos
                    pos += D
        xcol0[blk] = pos
        pos += b - a
        bounds.append(pos)
    ntot = pos

    # input chunk cuts: cfg["inchunks"] = block-group sizes (last padded)
    groups = list(cfg["inchunks"])
    cuts = [0]
    blk = 0
    for g in groups:
        blk = min(nblk, blk + g)
        cuts.append(bounds[blk])
        if blk == nblk:
            break
    if cuts[-1] != ntot:
        cuts.append(ntot)
    cuts = sorted(set(cuts))
    chunks = list(zip(cuts[:-1], cuts[1:]))

    return dict(npad=npad, offs=offs, grid=grid, pieces=pieces,
                row_pos=row_pos, xcol0=xcol0, chunks=chunks, ntot=ntot)


def build_nc(widths, cfg=None):
    cfg = cfg or CFG
    _install_tile_compat()
    pl = plan(widths, cfg)
    npad, ntot = pl["npad"], pl["ntot"]
    grid, pieces = pl["grid"], pl["pieces"]
    row_pos, xcol0 = pl["row_pos"], pl["xcol0"]

    nc = bass.Bass("TRN2", target_bir_lowering=False, debug=False)
    xin = nc.dram_tensor("xin", [P, ntot], F16, kind="ExternalInput").ap()
    y = nc.dram_tensor("y", [P, npad], F16, kind="ExternalOutput").ap()

    with tile.TileContext(nc) as tc, ExitStack() as ctx:
        const = ctx.enter_context(tc.tile_pool(name="const", bufs=1))
        psp = ctx.enter_context(tc.tile_pool(name="ps", bufs=5, space="PSUM"))

        Isb = const.tile([P, ntot], F16)
        Ysb = const.tile([P, npad], F16)

        dma_names = []
        for (a, b) in pl["chunks"]:
            inst = nc.sync.dma_start(Isb[:, a:b], xin[:, a:b])
            dma_names.append(inst.ins.name)

        if cfg["warmup"]:
            wpool = ctx.enter_context(tc.tile_pool(name="warm", bufs=1,
                                                   space="PSUM"))
            wsb = const.tile([P, D], F16, tag="warm")
            wps = wpool.tile([P, BLK], F32, space="PSUM", tag="warm")
            for i in range(cfg["warmup"]):
                nc.tensor.matmul(wps[:, (i % 4) * D:(i % 4) * D + D],
                                 wsb[:], wsb[:], start=True, stop=True)

        for blk, (a, b) in enumerate(grid):
            ps = psp.tile([P, b - a], F32, space="PSUM", tag="ps")
            for (c, e, s) in pieces:
                if c >= a and c < b:
                    rp = row_pos[s]
                    xp = xcol0[blk] + (c - a)
                    nc.tensor.matmul(ps[:, c - a:e - a],
                                     Isb[:, rp:rp + D],
                                     Isb[:, xp:xp + (e - c)],
                                     start=True, stop=True)
            if cfg["copy_mode"] == "half":
                frac = cfg["copy_frac"]
                if isinstance(frac, tuple):
                    frac = frac[blk] if blk < len(frac) else frac[-1]
                h = int(round((b - a) * frac))
                if blk == len(grid) - 1 and cfg["last_full_dve"]:
                    h = b - a
                if h > 0:
                    nc.vector.tensor_copy(out=Ysb[:, a:a + h], in_=ps[:, :h])
                if h < b - a:
                    nc.scalar.copy(Ysb[:, a + h:b], ps[:, h:])
            else:
                if blk % 2 == 0:
                    nc.vector.tensor_copy(out=Ysb[:, a:b], in_=ps[:])
                else:
                    nc.scalar.copy(Ysb[:, a:b], ps[:])

        # y out per group of grid blocks
        gi = 0
        blk = 0
        nblk = len(grid)
        while blk < nblk:
            n = cfg["y_groups"][gi] if gi < len(cfg["y_groups"]) else nblk - blk
            last = min(nblk, blk + n)
            a = grid[blk][0]
            b = grid[last - 1][1]
            eng = cfg["y_engines"][gi] if gi < len(cfg["y_engines"]) else "sync"
            getattr(nc, eng).dma_start(y[:, a:b], Ysb[:, a:b])
            blk = last
            gi += 1

    _split_multi_waits(nc)
    _hoist_before_entry_barrier(nc, dma_names, cfg["hoist"])
    return nc


_NC_CACHE = {}


def _pack_roundrobin(r, gcounts, n):
    """Deal each bucket round-robin across cores.  Slot s holds bucket
    order[s] on every core; per-core counts = ceil(global/8)."""
    bucket_w = -(-gcounts // N_CORES)
    order = np.argsort(-bucket_w, kind="stable")
    widths = tuple(int(bucket_w[k]) for k in order)
    slot_of = np.empty(NBUCKET, dtype=np.int64)
    slot_of[order] = np.arange(NBUCKET)
    offs = np.cumsum([0] + list(widths))

    gstart = np.zeros(NBUCKET, dtype=np.int64)
    np.cumsum(gcounts[:-1], out=gstart[1:])
    sort_idx = np.argsort(r, kind="stable")
    wrank = np.empty(n, dtype=np.int64)
    wrank[sort_idx] = np.arange(n) - np.repeat(gstart, gcounts)
    core = wrank % N_CORES
    pidx = offs[slot_of[r]] + wrank // N_CORES
    row_bucket = [[int(order[s]) for s in range(len(widths))]
                  for _ in range(N_CORES)]
    return widths, core, pidx, row_bucket


def _pack_twoslot(r, gcounts, n):
    """One whole small bucket per core + one chunk of the biggest bucket:
    only 2 table rows per core instead of 9."""
    big = int(np.argmax(gcounts))
    smalls = [k for k in range(NBUCKET) if k != big]
    w_small = int(max(gcounts[k] for k in smalls))
    w_big = int(-(-gcounts[big] // N_CORES))
    if w_small >= w_big:
        widths, s_small, s_big = (w_small, w_big), 0, 1
    else:
        widths, s_small, s_big = (w_big, w_small), 1, 0
    offs = np.cumsum([0] + list(widths))

    gstart = np.zeros(NBUCKET, dtype=np.int64)
    np.cumsum(gcounts[:-1], out=gstart[1:])
    sort_idx = np.argsort(r, kind="stable")
    wrank = np.empty(n, dtype=np.int64)
    wrank[sort_idx] = np.arange(n) - np.repeat(gstart, gcounts)

    core_of_small = np.zeros(NBUCKET, dtype=np.int64)
    for i, k in enumerate(smalls):
        core_of_small[k] = i
    is_big = r == big
    core = np.where(is_big, np.minimum(wrank // max(w_big, 1), N_CORES - 1),
                    core_of_small[r])
    pidx = np.where(is_big, offs[s_big] + wrank % max(w_big, 1),
                    offs[s_small] + wrank)
    row_bucket = []
    for c in range(N_CORES):
        rb = [None, None]
        rb[s_small] = smalls[c]
        rb[s_big] = big
        row_bucket.append(rb)
    return widths, core, pidx, row_bucket


def kernel(positions, outputs, table):
    positions = np.asarray(positions)
    outputs = np.asarray(outputs, dtype=np.float32)
    table = np.asarray(table, dtype=np.float32)
    T, B = positions.shape
    n = T * B

    r = np.where(positions < NBUCKET - 1, positions, NBUCKET - 1)
    r = np.mod(r, NBUCKET).astype(np.int64).reshape(n)  # table[] wraparound
    x = outputs.reshape(n, D)
    gcounts = np.bincount(r, minlength=NBUCKET)

    # pick the packing with the fewest DMA columns (x in + rows + y out)
    packs = [_pack_roundrobin(r, gcounts, n), _pack_twoslot(r, gcounts, n)]
    cost = [2 * sum(w) + D * sum(wi > 0 for wi in w)
            for (w, _, _, _) in packs]
    widths, core, pidx, row_bucket = packs[int(np.argmin(cost))]

    if widths not in _NC_CACHE:
        _NC_CACHE[widths] = build_nc(widths)
    nc = _NC_CACHE[widths]
    _NC_CACHE["nc"] = nc  # latest program, for the timing harness

    pl = plan(widths, CFG)
    row_pos, xcol0 = pl["row_pos"], pl["xcol0"]
    grid, ntot, npad = pl["grid"], pl["ntot"], pl["npad"]

    tbl_f16 = table.astype(np.float16)

    def xin_col(col):  # padded column -> concat column
        for blk, (a, b) in enumerate(grid):
            if col < b:
                return xcol0[blk] + (col - a)
        raise AssertionError

    ccol = np.array([xin_col(int(c)) for c in range(npad)], dtype=np.int64)

    in_maps = []
    masks = []
    for c in range(N_CORES):
        m = core == c
        masks.append(m)
        xin = np.zeros((P, ntot), dtype=np.float16)
        for s in range(len(widths)):
            if row_pos[s] is not None and row_bucket[c][s] is not None:
                xin[:, row_pos[s]:row_pos[s] + D] = tbl_f16[row_bucket[c][s]]
        xin[:, ccol[pidx[m]]] = x[m].astype(np.float16).T
        in_maps.append({"xin": xin})

    res = run_bass_kernel_spmd(nc, in_maps, list(range(N_CORES)))
    out = np.empty((n, D), dtype=np.float32)
    for c in range(N_CORES):
        out[masks[c]] = res.results[c]["y"][:, pidx[masks[c]]].T
    return out.reshape(T, B, D)


# revision 29
# speedup vs baseline: 1.0027x; 1.0027x over previous
"""MultiPositionTransfer kernel for 8 TRN2 NeuronCores (Bass/Tile).

Computes out[t,b,:] = outputs[t,b,:] @ table[min(positions[t,b], 8)] for
positions [512,32] int, outputs [512,32,128] f32, table [9,128,128] f32.
Sharding: data-parallel over T across 8 cores (2048 vectors per core);
the small table is replicated.

Per-core algorithm — count-specialized sorted GEMM:

The host buckets all T*B vectors by clipped position and deals each
bucket round-robin across the 8 cores, so per-core bucket counts are
ceil(global/8) and every core runs the same program (SPMD) with minimal
padding (npad <= 2048+9 columns for any input).  Each core's columns
are laid out slot-by-slot (buckets ordered big-first); the device runs
9 plain fp16 matmuls, one per slot, against the resident [d, 9*d]
table — no masks, no per-bucket elementwise work, no transposes (X is
sent host-transposed as [d, n]).  Slot matmuls are split on a
<=512-column PSUM-bank grid; each grid block accumulates in its own
rotating PSUM bank and is copied to SBUF fp16 by DVE/Act (alternating
whole blocks) before being DMAed out.  The host inverts the
permutation on the way back.

The table and X are concatenated into one DRAM input whose column
order follows first use (the rows a block newly needs precede its X
columns); the input chunk DMAs are hoisted to the very front of the SP
stream to hide the DMA front-end latency, and the teardown is trimmed
to the output-DMA completion waits (no drains / final barrier / sem
cleanup — the runtime resets semaphore state at NEFF load, verified on
hardware by the repeated-run test).  The schedule (input chunking,
PSUM grid, copy-engine split, output grouping/issue engines) is a
tunable CFG, auto-tuned against the instruction-cost timeline model.
The program depends only on the 9 slot widths; it is compiled once per
width tuple and cached (an unusual position distribution just triggers
one recompile and stays correct).
"""

import numpy as np
from contextlib import ExitStack

import concourse.bass as bass
import concourse.tile as tile
from concourse import mybir
from concourse.bass_utils import run_bass_kernel_spmd
from concourse.vector_clock import ScopedClock, VectorClock

P = 128
D = 128
NBUCKET = 9
F16 = mybir.dt.float16
F32 = mybir.dt.float32
N_CORES = 8
BLK = 512  # PSUM bank width in f32 columns

# schedule configuration (see tune.py)
CFG = dict(
    grid=("frac", (448, 512, 480, 480, 283)),  # PSUM grid block sizes
    inchunks=(2, 1, 1, 1),  # input DMA chunks as block-group sizes
    rows_mode="need",  # "need": rows before first-needing block; "front": all first
    copy_mode="half",  # "half": DVE+Act split; "alt": alternate full copies
    copy_frac=(1.0, 0.0, 1.0, 0.0, 1.0),  # DVE share per block copy
    last_full_dve=True,  # last block copied entirely by DVE
    hoist="front",     # hoist input DMAs: "barrier" | "front"
    y_groups=(1, 2, 2),       # blocks per y DMA, last group gets the rest
    y_engines=("scalar", "sync", "sync"),  # issue engine per y group
    warmup=0,          # dummy 128-col PE matmuls before the real work
)


def _drain_and_barrier_trimmed(self, tick_clock, wait_clock):
    nc = self.nc
    vec = tick_clock.global_clock
    for proc in range(len(vec)):
        if vec[proc] <= 0:
            continue
        unit = VectorClock([vec[p] if p == proc else 0 for p in range(len(vec))])
        nop_inst = nc.sync.nop()
        wait_clock.add_sem_waits(nop_inst.ins, ScopedClock({None: unit}))
    assert self.sems is not None
    popped = nc._tile_sem_poison_stack.pop()
    assert popped is self._sem_poison


def _install_tile_compat():
    tile.TileContext._drain_and_barrier = _drain_and_barrier_trimmed


def _split_multi_waits(nc):
    for fn in nc.m.functions:
        for bb in fn.blocks:
            insts = bb.instructions
            for i in range(len(insts) - 1, -1, -1):
                inst = insts[i]
                si = inst.sync_info
                if si is None:
                    continue
                waits = list(si.on_wait)
                cap = 0 if inst.opcode == "Drain" else 1
                if len(waits) <= cap:
                    continue
                keep = waits[len(waits) - cap:] if cap else []
                hoist = waits[: len(waits) - cap] if cap else waits
                nops = []
                for k, w in enumerate(hoist):
                    nops.append(mybir.InstNoOp(
                        name=f"{inst.name}-wsplit{k}",
                        engine=inst.engine,
                        sync_info=mybir.SyncInfo(on_wait=[w], on_update=[]),
                        bass_nofuse=True,
                    ))
                inst.sync_info = mybir.SyncInfo(
                    on_wait=keep, on_update=list(si.on_update))
                insts[i:i] = nops


def _hoist_before_entry_barrier(nc, inst_names, mode="barrier"):
    """Move the named (wait-free) instructions into the preamble block,
    just before the SP entry barrier."""
    targets = set(inst_names)
    fn = nc.m.functions[0]
    moved = []
    for bb in fn.blocks:
        insts = bb.instructions
        keep = []
        for inst in insts:
            if inst.name in targets:
                assert not (inst.sync_info and inst.sync_info.on_wait), inst.name
                moved.append(inst)
            else:
                keep.append(inst)
        if len(keep) != len(insts):
            insts[:] = keep
    assert len(moved) == len(targets), (len(moved), len(targets))
    bb0 = fn.blocks[0]
    bar = None
    for i, inst in enumerate(bb0.instructions):
        if mode == "front" and inst.engine == mybir.EngineType.SP:
            bar = i
            break
        if (inst.opcode == "EventSemaphore"
                and inst.engine == mybir.EngineType.SP):
            bar = i
            break
    assert bar is not None, "entry barrier not found"
    bb0.instructions[bar:bar] = moved


def plan(widths, cfg):
    """Shared host/device layout plan for a width tuple + schedule cfg."""
    widths = [int(w) for w in widths]
    npad = sum(widths)
    offs = np.cumsum([0] + widths)

    # PSUM grid
    if isinstance(cfg["grid"], tuple) and cfg["grid"][0] == "frac":
        # relative block sizes, scaled to npad, each capped at BLK
        rel = np.array(cfg["grid"][1], dtype=float)
        sizes = np.floor(rel / rel.sum() * npad).astype(int)
        sizes[-1] += npad - sizes.sum()
        while sizes.max() > BLK or sizes.min() <= 0:
            sizes = None
            break
        if sizes is None:
            sizes = []
            c = npad
            while c > 0:
                sizes.append(min(c, BLK))
                c -= min(c, BLK)
        cuts = np.cumsum([0] + list(sizes))
        grid = list(zip(cuts[:-1].tolist(), cuts[1:].tolist()))
    else:
        grid = []
        c = 0
        while c < npad:
            grid.append((c, min(npad, c + BLK)))
            c += BLK
        if isinstance(cfg["grid"], tuple) and cfg["grid"][0] == "tiny":
            t = cfg["grid"][1]
            a, b = grid[-1]
            if b - a > t:
                grid[-1] = (a, b - t)
                grid.append((b - t, b))

    # slot matmul pieces split on the grid
    nslot = len(widths)
    pieces = []
    for s in range(nslot):
        c, b = int(offs[s]), int(offs[s + 1])
        for (ga, gb) in grid:
            lo, hi = max(c, ga), min(b, gb)
            if lo < hi:
                pieces.append((lo, hi, s))

    nblk = len(grid)
    # concat layout: rows (per cfg) interleaved with per-block X columns
    row_pos = [None] * nslot
    xcol0 = [None] * nblk
    pos = 0
    if cfg["rows_mode"] == "front":
        for (c, e, s) in pieces:
            if row_pos[s] is None:
                row_pos[s] = pos
                pos += D
    bounds = [0]
    for blk, (a, b) in enumerate(grid):
        if cfg["rows_mode"] == "need":
            for (c, e, s) in pieces:
                if c >= a and c < b and row_pos[s] is None:
                    row_pos[s] = pos
                    pos += D
        xcol0[blk] = pos
        pos += b - a
        bounds.append(pos)
    ntot = pos

    # input chunk cuts: cfg["inchunks"] = block-group sizes (last padded)
    groups = list(cfg["inchunks"])
    cuts = [0]
    blk = 0
    for g in groups:
        blk = min(nblk, blk + g)
        cuts.append(bounds[blk])
        if blk == nblk:
            break
    if cuts[-1] != ntot:
        cuts.append(ntot)
    cuts = sorted(set(cuts))
    chunks = list(zip(cuts[:-1], cuts[1:]))

    return dict(npad=npad, offs=offs, grid=grid, pieces=pieces,
                row_pos=row_pos, xcol0=xcol0, chunks=chunks, ntot=ntot)


def build_nc(widths, cfg=None):
    cfg = cfg or CFG
    _install_tile_compat()
    pl = plan(widths, cfg)
    npad, ntot = pl["npad"], pl["ntot"]
    grid, pieces = pl["grid"], pl["pieces"]
    row_pos, xcol0 = pl["row_pos"], pl["xcol0"]

    nc = bass.Bass("TRN2", target_bir_lowering=False, debug=False)
    xin = nc.dram_tensor("xin", [P, ntot], F16, kind="ExternalInput").ap()
    y = nc.dram_tensor("y", [P, npad], F16, kind="ExternalOutput").ap()

    with tile.TileContext(nc) as tc, ExitStack() as ctx:
        const = ctx.enter_context(tc.tile_pool(name="const", bufs=1))
        psp = ctx.enter_context(tc.tile_pool(name="ps", bufs=5, space="PSUM"))

        Isb = const.tile([P, ntot], F16)
        Ysb = const.tile([P, npad], F16)

        dma_names = []
        for (a, b) in pl["chunks"]:
            inst = nc.sync.dma_start(Isb[:, a:b], xin[:, a:b])
            dma_names.append(inst.ins.name)

        if cfg["warmup"]:
            wpool = ctx.enter_context(tc.tile_pool(name="warm", bufs=1,
                                                   space="PSUM"))
            wsb = const.tile([P, D], F16, tag="warm")
            wps = wpool.tile([P, BLK], F32, space="PSUM", tag="warm")
            for i in range(cfg["warmup"]):
                nc.tensor.matmul(wps[:, (i % 4) * D:(i % 4) * D + D],
                                 wsb[:], wsb[:], start=True, stop=True)

        for blk, (a, b) in enumerate(grid):
            ps = psp.tile([P, b - a], F32, space="PSUM", tag="ps")
            for (c, e, s) in pieces:
                if c >= a and c < b:
                    rp = row_pos[s]
                    xp = xcol0[blk] + (c - a)
                    nc.tensor.matmul(ps[:, c - a:e - a],
                                     Isb[:, rp:rp + D],
                                     Isb[:, xp:xp + (e - c)],
                                     start=True, stop=True)
            if cfg["copy_mode"] == "half":
                frac = cfg["copy_frac"]
                if isinstance(frac, tuple):
                    frac = frac[blk] if blk < len(frac) else frac[-1]
                h = int(round((b - a) * frac))
                if blk == len(grid) - 1 and cfg["last_full_dve"]:
                    h = b - a
                if h > 0:
                    nc.vector.tensor_copy(out=Ysb[:, a:a + h], in_=ps[:, :h])
                if h < b - a:
                    nc.scalar.copy(Ysb[:, a + h:b], ps[:, h:])
            else:
                if blk % 2 == 0:
                    nc.vector.tensor_copy(out=Ysb[:, a:b], in_=ps[:])
                else:
                    nc.scalar.copy(Ysb[:, a:b], ps[:])

        # y out per group of grid blocks
        gi = 0
        blk = 0
        nblk = len(grid)
        while blk < nblk:
            n = cfg["y_groups"][gi] if gi < len(cfg["y_groups"]) else nblk - blk
            last = min(nblk, blk + n)
            a = grid[blk][0]
            b = grid[last - 1][1]
            eng = cfg["y_engines"][gi] if gi < len(cfg["y_engines"]) else "sync"
            getattr(nc, eng).dma_start(y[:, a:b], Ysb[:, a:b])
            blk = last
            gi += 1

    _split_multi_waits(nc)
    _hoist_before_entry_barrier(nc, dma_names, cfg["hoist"])
    return nc


_NC_CACHE = {}


def _pack_roundrobin(r, gcounts, n):
    """Deal each bucket round-robin across cores.  Slot s holds bucket
    order[s] on every core; per-core counts = ceil(global/8)."""
    bucket_w = -(-gcounts // N_CORES)
    order = np.argsort(-bucket_w, kind="stable")
    widths = tuple(int(bucket_w[k]) for k in order)
    slot_of = np.empty(NBUCKET, dtype=np.int64)
    slot_of[order] = np.arange(NBUCKET)
    offs = np.cumsum([0] + list(widths))

    gstart = np.zeros(NBUCKET, dtype=np.int64)
    np.cumsum(gcounts[:-1], out=gstart[1:])
    sort_idx = np.argsort(r, kind="stable")
    wrank = np.empty(n, dtype=np.int64)
    wrank[sort_idx] = np.arange(n) - np.repeat(gstart, gcounts)
    core = wrank % N_CORES
    pidx = offs[slot_of[r]] + wrank // N_CORES
    row_bucket = [[int(order[s]) for s in range(len(widths))]
                  for _ in range(N_CORES)]
    return widths, core, pidx, row_bucket


def _pack_twoslot(r, gcounts, n):
    """One whole small bucket per core + one chunk of the biggest bucket:
    only 2 table rows per core instead of 9."""
    big = int(np.argmax(gcounts))
    smalls = [k for k in range(NBUCKET) if k != big]
    w_small = int(max(gcounts[k] for k in smalls))
    w_big = int(-(-gcounts[big] // N_CORES))
    if w_small >= w_big:
        widths, s_small, s_big = (w_small, w_big), 0, 1
    else:
        widths, s_small, s_big = (w_big, w_small), 1, 0
    offs = np.cumsum([0] + list(widths))

    gstart = np.zeros(NBUCKET, dtype=np.int64)
    np.cumsum(gcounts[:-1], out=gstart[1:])
    sort_idx = np.argsort(r, kind="stable")
    wrank = np.empty(n, dtype=np.int64)
    wrank[sort_idx] = np.arange(n) - np.repeat(gstart, gcounts)

    core_of_small = np.zeros(NBUCKET, dtype=np.int64)
    for i, k in enumerate(smalls):
        core_of_small[k] = i
    is_big = r == big
    core = np.where(is_big, np.minimum(wrank // max(w_big, 1), N_CORES - 1),
                    core_of_small[r])
    pidx = np.where(is_big, offs[s_big] + wrank % max(w_big, 1),
                    offs[s_small] + wrank)
    row_bucket = []
    for c in range(N_CORES):
        rb = [None, None]
        rb[s_small] = smalls[c]
        rb[s_big] = big
        row_bucket.append(rb)
    return widths, core, pidx, row_bucket


def kernel(positions, outputs, table):
    positions = np.asarray(positions)
    outputs = np.asarray(outputs, dtype=np.float32)
    table = np.asarray(table, dtype=np.float32)
    T, B = positions.shape
    n = T * B

    r = np.where(positions < NBUCKET - 1, positions, NBUCKET - 1)
    r = np.mod(r, NBUCKET).astype(np.int64).reshape(n)  # table[] wraparound
    x = outputs.reshape(n, D)
    gcounts = np.bincount(r, minlength=NBUCKET)

    # pick the packing with the smallest padded width (the PSUM->SBUF copy
    # chain scales with npad and is the pipeline's binding constraint);
    # tie-break on total DMA columns
    packs = [_pack_roundrobin(r, gcounts, n), _pack_twoslot(r, gcounts, n)]
    cost = [(sum(w), 2 * sum(w) + D * sum(wi > 0 for wi in w))
            for (w, _, _, _) in packs]
    widths, core, pidx, row_bucket = packs[int(np.argmin(
        [c[0] * 100000 + c[1] for c in cost]))]

    if widths not in _NC_CACHE:
        _NC_CACHE[widths] = build_nc(widths)
    nc = _NC_CACHE[widths]
    _NC_CACHE["nc"] = nc  # latest program, for the timing harness

    pl = plan(widths, CFG)
    row_pos, xcol0 = pl["row_pos"], pl["xcol0"]
    grid, ntot, npad = pl["grid"], pl["ntot"], pl["npad"]

    tbl_f16 = table.astype(np.float16)

    def xin_col(col):  # padded column -> concat column
        for blk, (a, b) in enumerate(grid):
            if col < b:
                return xcol0[blk] + (col - a)
        raise AssertionError

    ccol = np.array([xin_col(int(c)) for c in range(npad)], dtype=np.int64)

    in_maps = []
    masks = []
    for c in range(N_CORES):
        m = core == c
        masks.append(m)
        xin = np.zeros((P, ntot), dtype=np.float16)
        for s in range(len(widths)):
            if row_pos[s] is not None and row_bucket[c][s] is not None:
                xin[:, row_pos[s]:row_pos[s] + D] = tbl_f16[row_bucket[c][s]]
        xin[:, ccol[pidx[m]]] = x[m].astype(np.float16).T
        in_maps.append({"xin": xin})

    res = run_bass_kernel_spmd(nc, in_maps, list(range(N_CORES)))
    out = np.empty((n, D), dtype=np.float32)
    for c in range(N_CORES):
        out[masks[c]] = res.results[c]["y"][:, pidx[masks[c]]].T
    return out.reshape(T, B, D)
